# revision 6
# baseline (speedup 1.0000x reference)
"""Multi-head causal self-attention (B=32, S=512, E=768, H=12, D=64) on 8 TRN2 cores.

Sharding: pure data-parallel over batch (4 batches per core), no collectives.

Per-core layout strategy:
  - x is fed pre-transposed (feature-major) as xT [E, 2048tok].
  - Q^T, K^T are computed feature-major:  QT = Wq.T @ xT   (lhsT=Wq, rhs=xT)
  - V is computed token-major with an extra all-ones column per head
    ("V_aug" [tok, H*(D+1)]); the ones column makes the P@V matmul also
    produce the softmax denominators.
  - scores^T[k,q] = K Q^T computed per (head, k-tile of 128 tokens) with the
    causal-trimmed q range [128*i, 512).  The diagonal 128x128 block gets the
    causal mask added in PSUM via an identity-matmul accumulation.
  - exp() on ScalarE reads PSUM directly (scale=1/8 folded into exp).
  - P@V: out[q, D+1] accumulated over k-tiles i<=j in PSUM; reciprocal of
    column D gives per-row normalization applied with tensor_scalar.
  - Y (token-major) is transposed 128x128 via TensorE back to feature-major
    for the output projection, which lands token-major for a contiguous DMA.
"""

import os
import sys

import numpy as np

for _p in ("/opt/trn_rl_repo", "/opt/trn_rl_repo/concourse"):
    if _p not in sys.path:
        sys.path.insert(0, _p)

import concourse.bass as bass
import concourse.bacc as bacc
import concourse.mybir as mybir
import concourse.tile as tile

P = 128
E = 768
S = 512
H = 12
D = 64
HP = H // 2          # head pairs
KT = E // P          # 6 feature k-tiles
N_CORES = 8
B_FULL = 32
B_CORE = B_FULL // N_CORES   # 4 batches per core
TOK = B_CORE * S             # 2048 tokens per core
ST = S // P                  # 4 token tiles per sequence
NEG = -1.0e6                 # pre-scale mask bias; exp(0.125 * -1e6) == 0
F32 = mybir.dt.float32

# number of 384-wide chunks for the V / O projections
CH = 2
CHW = E // CH  # 384


def build_program(with_bias: bool):
    nc = bacc.Bacc(None)

    xt_d = nc.dram_tensor("xt", [E, TOK], F32, kind="ExternalInput")
    w_d = {
        n: nc.dram_tensor(n, [E, E], F32, kind="ExternalInput")
        for n in ("wq", "wk", "wv", "wo")
    }
    consts_d = nc.dram_tensor("consts", [P, 2 * P], F32, kind="ExternalInput")
    if with_bias:
        # bq/bk laid out per-partition for the feature-major Q^T/K^T tiles;
        # bv in V_aug layout; bo broadcast for the final add.
        bqk_d = nc.dram_tensor("bqk", [P, 2 * KT], F32, kind="ExternalInput")
        bv_d = nc.dram_tensor("bvb", [P, H * (D + 1)], F32, kind="ExternalInput")
        bo_d = nc.dram_tensor("bob", [P, E], F32, kind="ExternalInput")
    y_d = nc.dram_tensor("y", [TOK, E], F32, kind="ExternalOutput")

    with tile.TileContext(nc) as tc:
        with (
            tc.tile_pool(name="wpool", bufs=1) as wpool,
            tc.tile_pool(name="xpool", bufs=2) as xpool,
            tc.tile_pool(name="qkpool", bufs=1) as qkpool,
            tc.tile_pool(name="vpool", bufs=1) as vpool,
            tc.tile_pool(name="ppool", bufs=10) as ppool,
            tc.tile_pool(name="ypool", bufs=4) as ypool,
            tc.tile_pool(name="ytpool", bufs=1) as ytpool,
            tc.tile_pool(name="opool", bufs=2) as opool,
            tc.tile_pool(name="rpool", bufs=4) as rpool,
            tc.tile_pool(name="ps_mm", bufs=2, space="PSUM") as ps_mm,
            tc.tile_pool(name="ps_sc", bufs=2, space="PSUM") as ps_sc,
            tc.tile_pool(name="ps_pv", bufs=2, space="PSUM") as ps_pv,
            tc.tile_pool(name="ps_yt", bufs=2, space="PSUM") as ps_yt,
        ):
            # ---- persistent constants ----
            w_sb = {}
            for n in ("wq", "wk", "wv", "wo"):
                t = wpool.tile([P, KT, E], F32, tag=n)
                nc.sync.dma_start(t[:], w_d[n][:].rearrange("(ko ki) m -> ki ko m", ki=P))
                w_sb[n] = t
            cons = wpool.tile([P, 2 * P], F32, tag="consts")
            nc.sync.dma_start(cons[:], consts_d[:])
            ident = cons[:, 0:P]
            maskb = cons[:, P : 2 * P]
            if with_bias:
                bqk = wpool.tile([P, 2 * KT], F32, tag="bqk")
                nc.sync.dma_start(bqk[:], bqk_d[:])
                bvb = wpool.tile([P, H * (D + 1)], F32, tag="bvb")
                nc.sync.dma_start(bvb[:], bv_d[:])
                bob = wpool.tile([P, E], F32, tag="bob")
                nc.sync.dma_start(bob[:], bo_d[:])

            xt_r = xt_d[:].rearrange("(ko ki) t -> ki ko t", ki=P)

            for b in range(B_CORE):
                tok0 = b * S

                # ---- load xT slice for this batch ----
                xts = xpool.tile([P, KT, S], F32, tag="xts")
                nc.sync.dma_start(xts[:], xt_r[:, :, tok0 : tok0 + S])

                # ---- Q^T and K^T projections (feature-major) ----
                qt = qkpool.tile([P, KT, S], F32, tag="qt")
                kt = qkpool.tile([P, KT, S], F32, tag="kt")
                for name, dst in (("wq", qt), ("wk", kt)):
                    for fo in range(KT):
                        ps = ps_mm.tile([P, S], F32, tag="mm")
                        for k in range(KT):
                            nc.tensor.matmul(
                                ps[:],
                                w_sb[name][:, k, fo * P : (fo + 1) * P],
                                xts[:, k, :],
                                start=(k == 0),
                                stop=(k == KT - 1),
                            )
                        if with_bias:
                            col = (0 if name == "wq" else KT) + fo
                            nc.vector.tensor_scalar_add(
                                dst[:, fo, :], ps[:], bqk[:, col : col + 1]
                            )
                        else:
                            nc.any.tensor_copy(out=dst[:, fo, :], in_=ps[:])

                # ---- V projection (token-major, augmented with ones cols) ----
                vs = []
                for tt in range(ST):
                    v_t = vpool.tile([P, H, D + 1], F32, tag=f"vs{tt}")
                    nc.gpsimd.memset(v_t[:, :, D : D + 1], 1.0)
                    for ch in range(CH):
                        ps = ps_mm.tile([P, S], F32, tag="mm")
                        psc = ps[:, :CHW]
                        for k in range(KT):
                            nc.tensor.matmul(
                                psc,
                                xts[:, k, tt * P : (tt + 1) * P],
                                w_sb["wv"][:, k, ch * CHW : (ch + 1) * CHW],
                                start=(k == 0),
                                stop=(k == KT - 1),
                            )
                        hpc = CHW // D  # heads per chunk (6)
                        dst = v_t[:, ch * hpc : (ch + 1) * hpc, 0:D]
                        nc.any.tensor_copy(out=dst, in_=psc.rearrange("p (h d) -> p h d", d=D))
                    if with_bias:
                        nc.vector.tensor_add(
                            out=v_t[:],
                            in0=v_t[:],
                            in1=bvb[:].rearrange("p (h d) -> p h d", d=D + 1),
                        )
                    vs.append(v_t)

                # ---- attention per head pair ----
                yt = ytpool.tile([P, KT, S], F32, tag="yt")
                for hp in range(HP):
                    # scores^T + exp, causal-trimmed per k-tile
                    pts = []  # pts[i][hh] = exp(scores^T) tile [P, Nq]
                    for i in range(ST):
                        nq = S - i * P
                        qoff = i * P
                        row = []
                        for hh in range(2):
                            ro = hh * D
                            ps = ps_sc.tile([P, S], F32, tag="sc")
                            nc.tensor.matmul(
                                ps[:, 0:nq],
                                kt[ro : ro + D, hp, i * P : (i + 1) * P],
                                qt[ro : ro + D, hp, qoff:S],
                                start=True,
                                stop=False,
                                tile_position=(ro, 0),
                            )
                            # add causal mask bias on the diagonal block
                            nc.tensor.matmul(
                                ps[:, 0:P],
                                ident,
                                maskb,
                                start=False,
                                stop=True,
                            )
                            pt = ppool.tile([P, S], F32, tag="pt")
                            nc.scalar.activation(
                                pt[:, 0:nq],
                                ps[:, 0:nq],
                                mybir.ActivationFunctionType.Exp,
                                scale=0.125,
                            )
                            row.append(pt)
                        pts.append(row)

                    # P @ V_aug accumulated over k-tiles, then normalize,
                    # then transpose Y back to feature-major.
                    for j in range(ST):
                        yst = ypool.tile([P, P], F32, tag="yst")
                        for hh in range(2):
                            h = 2 * hp + hh
                            pv = ps_pv.tile([P, D + 1], F32, tag="pv")
                            for i in range(j + 1):
                                nc.tensor.matmul(
                                    pv[:],
                                    pts[i][hh][:, (j - i) * P : (j - i + 1) * P],
                                    vs[i][:, h, :],
                                    start=(i == 0),
                                    stop=(i == j),
                                )
                            r = rpool.tile([P, 1], F32, tag="r")
                            nc.vector.reciprocal(r[:], pv[:, D : D + 1])
                            nc.vector.tensor_scalar_mul(
                                yst[:, hh * D : (hh + 1) * D], pv[:, 0:D], r[:]
                            )
                        yt_ps = ps_yt.tile([P, P], F32, tag="ytp")
                        nc.tensor.transpose(yt_ps[:], yst[:], ident)
                        nc.any.tensor_copy(
                            out=yt[:, hp, j * P : (j + 1) * P], in_=yt_ps[:]
                        )

                # ---- output projection ----
                for tt in range(ST):
                    o_sb = opool.tile([P, E], F32, tag="osb")
                    for ch in range(CH):
                        ps = ps_mm.tile([P, S], F32, tag="mm")
                        psc = ps[:, :CHW]
                        for k in range(KT):
                            nc.tensor.matmul(
                                psc,
                                yt[:, k, tt * P : (tt + 1) * P],
                                w_sb["wo"][:, k, ch * CHW : (ch + 1) * CHW],
                                start=(k == 0),
                                stop=(k == KT - 1),
                            )
                        nc.any.tensor_copy(
                            out=o_sb[:, ch * CHW : (ch + 1) * CHW], in_=psc
                        )
                    if with_bias:
                        nc.vector.tensor_add(out=o_sb[:], in0=o_sb[:], in1=bob[:])
                    nc.sync.dma_start(
                        y_d[tok0 + tt * P : tok0 + (tt + 1) * P, :], o_sb[:]
                    )

    nc.compile()
    return nc


def _host_consts():
    ident = np.eye(P, dtype=np.float32)
    k_idx = np.arange(P, dtype=np.int64)[:, None]
    q_idx = np.arange(P, dtype=np.int64)[None, :]
    maskb = np.where(k_idx <= q_idx, 0.0, NEG).astype(np.float32)
    return np.concatenate([ident, maskb], axis=1)  # [P, 2P]


_PROG_CACHE = {}


def _get_program(with_bias: bool):
    if with_bias not in _PROG_CACHE:
        _PROG_CACHE[with_bias] = build_program(with_bias)
    return _PROG_CACHE[with_bias]


def make_in_maps(x, Wq, bq, Wk, bk, Wv, bv, Wo, bo, with_bias):
    consts = _host_consts()
    maps = []
    for c in range(N_CORES):
        xc = np.ascontiguousarray(
            x[c * B_CORE : (c + 1) * B_CORE]  # [B_CORE, S, E]
            .reshape(TOK, E)
            .T  # [E, TOK]
        ).astype(np.float32)
        m = {
            "xt": xc,
            "wq": np.ascontiguousarray(Wq, dtype=np.float32),
            "wk": np.ascontiguousarray(Wk, dtype=np.float32),
            "wv": np.ascontiguousarray(Wv, dtype=np.float32),
            "wo": np.ascontiguousarray(Wo, dtype=np.float32),
            "consts": consts,
        }
        if with_bias:
            # bq/bk per-partition layout matching qt/kt tiles: [P, 2*KT]
            bqk = np.stack(
                [b.reshape(KT, P).T for b in (bq, bk)], axis=-1
            )  # [P, KT, 2] -> need [P, 2*KT] with col = which*KT + fo
            bqk = np.concatenate(
                [bq.reshape(KT, P).T, bk.reshape(KT, P).T], axis=1
            ).astype(np.float32)
            bvb = np.zeros((P, H, D + 1), np.float32)
            bvb[:, :, :D] = np.broadcast_to(bv.reshape(H, D), (P, H, D))
            m["bqk"] = np.ascontiguousarray(bqk)
            m["bvb"] = np.ascontiguousarray(bvb.reshape(P, H * (D + 1)))
            m["bob"] = np.ascontiguousarray(
                np.broadcast_to(bo.astype(np.float32), (P, E))
            )
        maps.append(m)
    return maps


def kernel(x, Wq, bq, Wk, bk, Wv, bv, Wo, bo):
    from concourse.bass_utils import run_bass_kernel_spmd

    x = np.asarray(x, dtype=np.float32)
    with_bias = any(
        float(np.abs(np.asarray(b)).max()) != 0.0 for b in (bq, bk, bv, bo)
    )
    nc = _get_program(with_bias)
    in_maps = make_in_maps(x, Wq, bq, Wk, bk, Wv, bv, Wo, bo, with_bias)
    res = run_bass_kernel_spmd(nc, in_maps, core_ids=list(range(N_CORES)))
    out = np.empty((B_FULL, S, E), dtype=np.float32)
    for c in range(N_CORES):
        out[c * B_CORE : (c + 1) * B_CORE] = res.results[c]["y"].reshape(B_CORE, S, E)
    return out


# revision 8
# speedup vs baseline: 609.4982x; 609.4982x over previous
"""Multi-head causal self-attention (B=32, S=512, E=768, H=12, D=64) on 8 TRN2 cores.

Sharding: pure data-parallel over batch (4 batches per core), no collectives.

Per-core layout strategy:
  - x is fed pre-transposed (feature-major) as xT [E, 2048tok].
  - Q^T, K^T are computed feature-major:  QT = Wq.T @ xT   (lhsT=Wq, rhs=xT)
  - V is computed token-major with an extra all-ones column per head
    ("V_aug" [tok, H*(D+1)]); the ones column makes the P@V matmul also
    produce the softmax denominators.
  - scores^T[k,q] = K Q^T computed per (head, k-tile of 128 tokens) with the
    causal-trimmed q range [128*i, 512).  The diagonal 128x128 block gets the
    causal mask added in PSUM via an identity-matmul accumulation.
  - exp() on ScalarE reads PSUM directly (scale=1/8 folded into exp).
  - P@V: out[q, D+1] accumulated over k-tiles i<=j in PSUM; reciprocal of
    column D gives per-row normalization applied with tensor_scalar.
  - Y (token-major) is transposed 128x128 via TensorE back to feature-major
    for the output projection, which lands token-major for a contiguous DMA.
"""

import os
import sys

import numpy as np

for _p in ("/opt/trn_rl_repo", "/opt/trn_rl_repo/concourse"):
    if _p not in sys.path:
        sys.path.insert(0, _p)

import concourse.bass as bass
import concourse.bacc as bacc
import concourse.mybir as mybir
import concourse.tile as tile

P = 128
E = 768
S = 512
H = 12
D = 64
HP = H // 2          # head pairs
KT = E // P          # 6 feature k-tiles
N_CORES = 8
B_FULL = 32
B_CORE = B_FULL // N_CORES   # 4 batches per core
TOK = B_CORE * S             # 2048 tokens per core
ST = S // P                  # 4 token tiles per sequence
NEG = -1.0e6                 # pre-scale mask bias; exp(0.125 * -1e6) == 0
F32 = mybir.dt.float32

# number of 384-wide chunks for the V / O projections
CH = 2
CHW = E // CH  # 384


def build_program(with_bias: bool, repeat: int = 1):
    nc = bacc.Bacc(None)

    xt_d = nc.dram_tensor("xt", [E, TOK], F32, kind="ExternalInput")
    w_d = {
        n: nc.dram_tensor(n, [E, E], F32, kind="ExternalInput")
        for n in ("wq", "wk", "wv", "wo")
    }
    consts_d = nc.dram_tensor("consts", [P, 2 * P], F32, kind="ExternalInput")
    if with_bias:
        # bq/bk laid out per-partition for the feature-major Q^T/K^T tiles;
        # bv in V_aug layout; bo broadcast for the final add.
        bqk_d = nc.dram_tensor("bqk", [P, 2 * KT], F32, kind="ExternalInput")
        bv_d = nc.dram_tensor("bvb", [P, H * (D + 1)], F32, kind="ExternalInput")
        bo_d = nc.dram_tensor("bob", [P, E], F32, kind="ExternalInput")
    y_d = nc.dram_tensor("y", [TOK, E], F32, kind="ExternalOutput")

    with tile.TileContext(nc) as tc:
        with (
            tc.tile_pool(name="wpool", bufs=1) as wpool,
            tc.tile_pool(name="xpool", bufs=2) as xpool,
            tc.tile_pool(name="qkpool", bufs=1) as qkpool,
            tc.tile_pool(name="vpool", bufs=1) as vpool,
            tc.tile_pool(name="ppool", bufs=10) as ppool,
            tc.tile_pool(name="ypool", bufs=4) as ypool,
            tc.tile_pool(name="ytpool", bufs=1) as ytpool,
            tc.tile_pool(name="opool", bufs=2) as opool,
            tc.tile_pool(name="rpool", bufs=4) as rpool,
            tc.tile_pool(name="ps_mm", bufs=2, space="PSUM") as ps_mm,
            tc.tile_pool(name="ps_sc", bufs=2, space="PSUM") as ps_sc,
            tc.tile_pool(name="ps_pv", bufs=2, space="PSUM") as ps_pv,
            tc.tile_pool(name="ps_yt", bufs=2, space="PSUM") as ps_yt,
        ):
            # ---- persistent constants ----
            w_sb = {}
            for n in ("wq", "wk", "wv", "wo"):
                t = wpool.tile([P, KT, E], F32, tag=n)
                nc.sync.dma_start(t[:], w_d[n][:].rearrange("(ko ki) m -> ki ko m", ki=P))
                w_sb[n] = t
            cons = wpool.tile([P, 2 * P], F32, tag="consts")
            nc.sync.dma_start(cons[:], consts_d[:])
            ident = cons[:, 0:P]
            maskb = cons[:, P : 2 * P]
            if with_bias:
                bqk = wpool.tile([P, 2 * KT], F32, tag="bqk")
                nc.sync.dma_start(bqk[:], bqk_d[:])
                bvb = wpool.tile([P, H * (D + 1)], F32, tag="bvb")
                nc.sync.dma_start(bvb[:], bv_d[:])
                bob = wpool.tile([P, E], F32, tag="bob")
                nc.sync.dma_start(bob[:], bo_d[:])

            xt_r = xt_d[:].rearrange("(ko ki) t -> ki ko t", ki=P)

            for b in range(B_CORE * repeat):
                b = b % B_CORE
                tok0 = b * S

                # ---- load xT slice for this batch ----
                xts = xpool.tile([P, KT, S], F32, tag="xts")
                nc.sync.dma_start(xts[:], xt_r[:, :, tok0 : tok0 + S])

                # ---- Q^T and K^T projections (feature-major) ----
                qt = qkpool.tile([P, KT, S], F32, tag="qt")
                kt = qkpool.tile([P, KT, S], F32, tag="kt")
                for name, dst in (("wq", qt), ("wk", kt)):
                    for fo in range(KT):
                        ps = ps_mm.tile([P, S], F32, tag="mm")
                        for k in range(KT):
                            nc.tensor.matmul(
                                ps[:],
                                w_sb[name][:, k, fo * P : (fo + 1) * P],
                                xts[:, k, :],
                                start=(k == 0),
                                stop=(k == KT - 1),
                            )
                        if with_bias:
                            col = (0 if name == "wq" else KT) + fo
                            nc.vector.tensor_scalar_add(
                                dst[:, fo, :], ps[:], bqk[:, col : col + 1]
                            )
                        else:
                            nc.any.tensor_copy(out=dst[:, fo, :], in_=ps[:])

                # ---- V projection (token-major, augmented with ones cols) ----
                vs = []
                for tt in range(ST):
                    v_t = vpool.tile([P, H, D + 1], F32, tag=f"vs{tt}")
                    nc.gpsimd.memset(v_t[:, :, D : D + 1], 1.0)
                    for ch in range(CH):
                        ps = ps_mm.tile([P, S], F32, tag="mm")
                        psc = ps[:, :CHW]
                        for k in range(KT):
                            nc.tensor.matmul(
                                psc,
                                xts[:, k, tt * P : (tt + 1) * P],
                                w_sb["wv"][:, k, ch * CHW : (ch + 1) * CHW],
                                start=(k == 0),
                                stop=(k == KT - 1),
                            )
                        hpc = CHW // D  # heads per chunk (6)
                        dst = v_t[:, ch * hpc : (ch + 1) * hpc, 0:D]
                        nc.any.tensor_copy(out=dst, in_=psc.rearrange("p (h d) -> p h d", d=D))
                    if with_bias:
                        nc.vector.tensor_add(
                            out=v_t[:],
                            in0=v_t[:],
                            in1=bvb[:].rearrange("p (h d) -> p h d", d=D + 1),
                        )
                    vs.append(v_t)

                # ---- attention per head pair ----
                yt = ytpool.tile([P, KT, S], F32, tag="yt")
                for hp in range(HP):
                    # scores^T + exp, causal-trimmed per k-tile
                    pts = []  # pts[i][hh] = exp(scores^T) tile [P, Nq]
                    for i in range(ST):
                        nq = S - i * P
                        qoff = i * P
                        row = []
                        for hh in range(2):
                            ro = hh * D
                            ps = ps_sc.tile([P, S], F32, tag="sc")
                            nc.tensor.matmul(
                                ps[:, 0:nq],
                                kt[ro : ro + D, hp, i * P : (i + 1) * P],
                                qt[ro : ro + D, hp, qoff:S],
                                start=True,
                                stop=False,
                                tile_position=(ro, 0),
                            )
                            # add causal mask bias on the diagonal block
                            nc.tensor.matmul(
                                ps[:, 0:P],
                                ident,
                                maskb,
                                start=False,
                                stop=True,
                            )
                            pt = ppool.tile([P, S], F32, tag="pt")
                            nc.scalar.activation(
                                pt[:, 0:nq],
                                ps[:, 0:nq],
                                mybir.ActivationFunctionType.Exp,
                                scale=0.125,
                            )
                            row.append(pt)
                        pts.append(row)

                    # P @ V_aug accumulated over k-tiles, then normalize,
                    # then transpose Y back to feature-major.
                    for j in range(ST):
                        yst = ypool.tile([P, P], F32, tag="yst")
                        for hh in range(2):
                            h = 2 * hp + hh
                            pv = ps_pv.tile([P, D + 1], F32, tag="pv")
                            for i in range(j + 1):
                                nc.tensor.matmul(
                                    pv[:],
                                    pts[i][hh][:, (j - i) * P : (j - i + 1) * P],
                                    vs[i][:, h, :],
                                    start=(i == 0),
                                    stop=(i == j),
                                )
                            r = rpool.tile([P, 1], F32, tag="r")
                            nc.vector.reciprocal(r[:], pv[:, D : D + 1])
                            nc.vector.tensor_scalar_mul(
                                yst[:, hh * D : (hh + 1) * D], pv[:, 0:D], r[:]
                            )
                        yt_ps = ps_yt.tile([P, P], F32, tag="ytp")
                        nc.tensor.transpose(yt_ps[:], yst[:], ident)
                        nc.any.tensor_copy(
                            out=yt[:, hp, j * P : (j + 1) * P], in_=yt_ps[:]
                        )

                # ---- output projection ----
                for tt in range(ST):
                    o_sb = opool.tile([P, E], F32, tag="osb")
                    for ch in range(CH):
                        ps = ps_mm.tile([P, S], F32, tag="mm")
                        psc = ps[:, :CHW]
                        for k in range(KT):
                            nc.tensor.matmul(
                                psc,
                                yt[:, k, tt * P : (tt + 1) * P],
                                w_sb["wo"][:, k, ch * CHW : (ch + 1) * CHW],
                                start=(k == 0),
                                stop=(k == KT - 1),
                            )
                        nc.any.tensor_copy(
                            out=o_sb[:, ch * CHW : (ch + 1) * CHW], in_=psc
                        )
                    if with_bias:
                        nc.vector.tensor_add(out=o_sb[:], in0=o_sb[:], in1=bob[:])
                    nc.sync.dma_start(
                        y_d[tok0 + tt * P : tok0 + (tt + 1) * P, :], o_sb[:]
                    )

    nc.compile()
    return nc


def _host_consts():
    ident = np.eye(P, dtype=np.float32)
    k_idx = np.arange(P, dtype=np.int64)[:, None]
    q_idx = np.arange(P, dtype=np.int64)[None, :]
    maskb = np.where(k_idx <= q_idx, 0.0, NEG).astype(np.float32)
    return np.concatenate([ident, maskb], axis=1)  # [P, 2P]


_PROG_CACHE = {}


def _get_program(with_bias: bool):
    if with_bias not in _PROG_CACHE:
        _PROG_CACHE[with_bias] = build_program(with_bias)
    return _PROG_CACHE[with_bias]


def make_in_maps(x, Wq, bq, Wk, bk, Wv, bv, Wo, bo, with_bias):
    consts = _host_consts()
    maps = []
    for c in range(N_CORES):
        xc = np.ascontiguousarray(
            x[c * B_CORE : (c + 1) * B_CORE]  # [B_CORE, S, E]
            .reshape(TOK, E)
            .T  # [E, TOK]
        ).astype(np.float32)
        m = {
            "xt": xc,
            "wq": np.ascontiguousarray(Wq, dtype=np.float32),
            "wk": np.ascontiguousarray(Wk, dtype=np.float32),
            "wv": np.ascontiguousarray(Wv, dtype=np.float32),
            "wo": np.ascontiguousarray(Wo, dtype=np.float32),
            "consts": consts,
        }
        if with_bias:
            # bq/bk per-partition layout matching qt/kt tiles: [P, 2*KT]
            bqk = np.stack(
                [b.reshape(KT, P).T for b in (bq, bk)], axis=-1
            )  # [P, KT, 2] -> need [P, 2*KT] with col = which*KT + fo
            bqk = np.concatenate(
                [bq.reshape(KT, P).T, bk.reshape(KT, P).T], axis=1
            ).astype(np.float32)
            bvb = np.zeros((P, H, D + 1), np.float32)
            bvb[:, :, :D] = np.broadcast_to(bv.reshape(H, D), (P, H, D))
            m["bqk"] = np.ascontiguousarray(bqk)
            m["bvb"] = np.ascontiguousarray(bvb.reshape(P, H * (D + 1)))
            m["bob"] = np.ascontiguousarray(
                np.broadcast_to(bo.astype(np.float32), (P, E))
            )
        maps.append(m)
    return maps


def kernel(x, Wq, bq, Wk, bk, Wv, bv, Wo, bo):
    from concourse.bass_utils import run_bass_kernel_spmd

    x = np.asarray(x, dtype=np.float32)
    with_bias = any(
        float(np.abs(np.asarray(b)).max()) != 0.0 for b in (bq, bk, bv, bo)
    )
    nc = _get_program(with_bias)
    in_maps = make_in_maps(x, Wq, bq, Wk, bk, Wv, bv, Wo, bo, with_bias)
    res = run_bass_kernel_spmd(nc, in_maps, core_ids=list(range(N_CORES)))
    out = np.empty((B_FULL, S, E), dtype=np.float32)
    for c in range(N_CORES):
        out[c * B_CORE : (c + 1) * B_CORE] = res.results[c]["y"].reshape(B_CORE, S, E)
    return out
